# revision 74
# baseline (speedup 1.0000x reference)
"""Trainium2 Bass kernel for nn_AttentionToVec (B=8, N=4096, E=1024, H=16, D=64).

Strategy: data-parallel over batch (1 batch element per NeuronCore) for the
attention part; tensor-parallel over the MLP hidden dim within groups of
TP_G=4 cores (collective latency scales with peer count, so 2 groups of 4
beat 1 group of 8) with an AllGather of the per-core sampled vectors and a
ReduceScatter of the partial MLP outputs (which lands exactly each core's own
output row).

Algebraic restructuring (host does weight-only folding):
  - att logits = x @ w_att where w_att[e,h] = sum_d W_k[e, h*D+d] * query[h,d]
    (the k-projection bias cancels inside softmax over n).
  - yhat[h,:] = sum_n exp_att[n,h] * x[n,:]  (deferred 1/Z normalization)
  - sampled[h,d] = (yhat[h,:] @ W_v[:, h*D+d])/Z + b_v[h*D+d]
  - Z comes from the Exp activation's accum_out minus the host-counted
    masked-row count (masked x rows are zeroed on host => exp(0)=1 each).

Layout: x is loaded from HBM twice, in both layouts, as bf16; all tensors are
host-prepped in their exact on-chip layouts so every DMA is one contiguous
run per partition (descriptor-light triggers). Small transposes (softmax
weights, y, S, h) are single DMA XBAR transposes (2-byte dtype; ~1.1us fixed
cost each, so batch them). W1/W2 load into the SBUF space the x stream frees
after phase B, with triggers that fire only then (no bus contention with x).
"""

import numpy as np

B = 8
N = 4096
E = 1024
H = 16
D = 64
HID = 4096
NCORES = 8
TP_G = 4  # MLP tensor-parallel group size
HID_C = HID // TP_G
NT = N // 128  # 32 n-tiles

_CACHE = {}

# Native HW gelu LUT vs a 5-op sigmoid-identity chain (exact same tanh-approx
# formula; CoreSim only implements the chain path).
GELU_NATIVE = True


def _np_bf16():
    import ml_dtypes

    return np.dtype(ml_dtypes.bfloat16)


def _np_f8():
    import ml_dtypes

    return np.dtype(ml_dtypes.float8_e4m3fn)


def _build():
    import concourse.bacc as bacc
    import concourse.mybir as mybir
    from concourse import tile
    import concourse.bass as bass_mod

    f32 = mybir.dt.float32
    bf16 = mybir.dt.bfloat16
    Act = mybir.ActivationFunctionType
    Alu = mybir.AluOpType

    nc = bacc.Bacc(None, target_bir_lowering=False, debug=True, num_devices=NCORES)

    f8 = mybir.dt.float8e4
    xn = nc.dram_tensor("xn", [128, NT, E], bf16, kind="ExternalInput")
    # phase A runs in fp8(e4m3) with DoubleRow (2 contraction rows/cycle):
    # w_att is host-scaled by 256 (its values sit below e4m3's normal range),
    # compensated exactly via the Exp activation's scale=1/256.
    xT = nc.dram_tensor("xT", [E, N], f8, kind="ExternalInput")
    wattc = nc.dram_tensor("wattc", [128, 4, 2, H], f8, kind="ExternalInput")
    Wv = nc.dram_tensor("Wv", [128, 8, E], bf16, kind="ExternalInput")
    # packed f32 consts: [bvb | nmask | b1c | b2r]
    cpack = nc.dram_tensor(
        "cpack", [H, E + 1 + HID_C + E], f32, kind="ExternalInput"
    )
    W1c = nc.dram_tensor("W1c", [128, 8, HID_C], bf16, kind="ExternalInput")
    W2c = nc.dram_tensor("W2c", [128, HID_C // 128, E], bf16, kind="ExternalInput")
    out = nc.dram_tensor("out", [1, E], f32, kind="ExternalOutput")

    with tile.TileContext(nc) as tc:
        with (
            tc.tile_pool(name="consts", bufs=1) as consts,
            tc.tile_pool(name="xtp", bufs=8) as xtp,
            tc.tile_pool(name="wp", bufs=1) as wp,
            tc.tile_pool(name="work", bufs=1) as work,
            tc.tile_pool(name="dramp", bufs=1, space="DRAM") as dramp,
        ):
            # ---- constant loads (one packed f32 DMA + the fp8 watt) ----
            wattc_s = consts.tile([128, 4, 2, H], f8)
            nc.sync.dma_start(out=wattc_s[:], in_=wattc[:, :, :, :])
            cpk = consts.tile([H, E + 1 + HID_C + E], f32)
            nc.sync.dma_start(out=cpk[:], in_=cpack[:, :])
            bvb_s = cpk[:, 0:E]
            nmask_s = cpk[:, E : E + 1]
            b1_s = cpk[0:TP_G, E + 1 : E + 1 + HID_C]
            b28_s = cpk[0:TP_G, E + 1 + HID_C :]

            # S16/hh16 rows past the data are read by the XBAR transposes but
            # their transposed columns are never consumed; zero once up front.
            S16 = work.tile([H, E], bf16, tag="S16")
            nc.vector.memset(S16[:], 0.0)
            hh16 = work.tile([H, HID_C], bf16, tag="hh16")
            nc.vector.memset(hh16[:], 0.0)
            # warm the exp table off the critical path
            warm = work.tile([H, 1], f32, tag="warm")
            nc.scalar.activation(warm[:], nmask_s[:], Act.Exp)

            # ---- x streams on the sync HWDGE queues; Wv on scalar's ----
            # xT in fp8 as 4 double-chunks: xt[p, i, n] = x[n, c*256 + i*128 + p]
            xtp_tiles = []
            xTr = xT.ap().rearrange("(c i p) n -> c p i n", i=2, p=128)
            for c in range(4):
                xt = xtp.tile([128, 2, N], f8, tag="xT")
                nc.sync.dma_start(out=xt[:], in_=xTr[c])
                xtp_tiles.append(xt)

            xsp_cm = tc.tile_pool(name="xsp", bufs=1)
            xsp = xsp_cm.__enter__()
            xs = xsp.tile([128, NT, E], bf16)
            for k in range(8):
                nc.sync.dma_start(
                    out=xs[:, 4 * k : 4 * k + 4, :], in_=xn[:, 4 * k : 4 * k + 4, :]
                )

            wv_s = wp.tile([128, 8, E], bf16, tag="wv")
            nc.scalar.dma_start(out=wv_s[:], in_=Wv[:, :, :])

            # ---- Phase A: attT[16, N] = w_att^T @ x^T (fp8 DoubleRow) ----
            # Block-outer over four 1024-wide blocks, each in its own 2-bank
            # PSUM tile, so exp g + attn-XBAR g pipeline with later blocks'
            # matmuls and phase B can start after block 0's XBAR.
            expm = work.tile([H, N], bf16)
            zparts = work.tile([H, 2], f32)
            attn = work.tile([128, NT, H], bf16)
            psA_cm = tc.tile_pool(name="psA", bufs=2, space="PSUM")
            psA = psA_cm.__enter__()
            for g in range(2):
                att_g = psA.tile([H, 2048], f32, tag="attT")
                for c in range(4):
                    xt = xtp_tiles[c]
                    for j in range(4):
                        nc.tensor.matmul(
                            att_g[:, 512 * j : 512 * (j + 1)],
                            wattc_s[:, c, :, :],
                            xt[:, :, 2048 * g + 512 * j : 2048 * g + 512 * (j + 1)],
                            start=(c == 0),
                            stop=(c == 3),
                            perf_mode=mybir.MatmulPerfMode.DoubleRow,
                        )
                sl = slice(2048 * g, 2048 * (g + 1))
                # exp (also the PSUM->SBUF move); scale undoes the w_att x256;
                # accum_out produces this block's Z contribution.
                nc.scalar.activation(
                    expm[:, sl],
                    att_g[:],
                    Act.Exp,
                    scale=1.0 / 256.0,
                    accum_out=zparts[:, g : g + 1],
                )
                # attn[p, t, h] = expm[h, t*128+p]
                nc.scalar.dma_start(
                    out=attn[:, 16 * g : 16 * (g + 1), :],
                    in_=expm[:, sl],
                    transpose=True,
                )
            psA_cm.__exit__(None, None, None)

            # Z = sum parts - (#masked rows); 1/Z  (off critical path, vector)
            zsum = work.tile([H, 1], f32)
            nc.vector.tensor_reduce(zsum[:], zparts[:], mybir.AxisListType.X, Alu.add)
            zc = work.tile([H, 1], f32)
            nc.vector.tensor_sub(zc[:], zsum[:], nmask_s[:])
            rz = work.tile([H, 1], f32)
            nc.vector.reciprocal(rz[:], zc[:])

            # ---- Phase B: yhat[16, E] = exp_att^T @ x (accumulate over n) ----
            psB_cm = tc.tile_pool(name="psB", bufs=1, space="PSUM")
            psB = psB_cm.__enter__()
            y_ps = psB.tile([H, E], f32)
            for t in range(NT):
                lhs = attn[:, t, :]
                nc.tensor.matmul(
                    y_ps[:, 0:512], lhs, xs[:, t, 0:512], start=(t == 0), stop=(t == NT - 1)
                )
                nc.tensor.matmul(
                    y_ps[:, 512:1024], lhs, xs[:, t, 512:1024], start=(t == 0), stop=(t == NT - 1)
                )
            # PSUM->SBUF move on scalar: the yT XBAR (also scalar) then needs
            # no cross-engine semaphore at all.
            y_sb = work.tile([H, E], bf16)
            nc.scalar.activation(y_sb[:], y_ps[:], Act.Copy)
            psB_cm.__exit__(None, None, None)
            xsp_cm.__exit__(None, None, None)

            # yT[p, c, h] = yhat[h, c*128+p] — issued BEFORE the W1/W2 triggers
            # so its descriptors aren't stuck behind 4MB on scalar's queues
            yT = work.tile([128, 8, H], bf16)
            nc.scalar.dma_start(out=yT[:], in_=y_sb[:], transpose=True)
            if GELU_NATIVE:
                # warm the gelu table now; the read of y_sb (value unused) is
                # an artificial dep so the scheduler cannot hoist this before
                # the exps (single table slot — hoisting would waste it).
                nc.scalar.activation(warm[:], y_sb[:, 0:1], Act.Gelu_apprx_tanh)

            # W1/W2 allocate in the space xs freed; their triggers sit AFTER
            # the yT XBAR on the scalar engine so their 4MB of descriptors
            # can never delay it (XBARs queue FIFO behind bulk DMAs).
            wl_cm = tc.tile_pool(name="wl", bufs=1)
            wl = wl_cm.__enter__()
            w1_s = wl.tile([128, 8, HID_C], bf16, tag="w1")
            nc.scalar.dma_start(out=w1_s[:], in_=W1c[:, :, :])
            w2_s = wl.tile([128, HID_C // 128, E], bf16, tag="w2")
            nc.scalar.dma_start(out=w2_s[:], in_=W2c[:, :, :])

            # ---- Phase C: sf = yhat @ Wv; sampled = diag(sf)*rz + bv ----
            psC_cm = tc.tile_pool(name="psC", bufs=1, space="PSUM")
            psC = psC_cm.__enter__()
            sf_ps = psC.tile([H, E], f32)
            for c in range(8):
                for j in range(2):
                    nc.tensor.matmul(
                        sf_ps[:, 512 * j : 512 * (j + 1)],
                        yT[:, c, :],
                        wv_s[:, c, 512 * j : 512 * (j + 1)],
                        start=(c == 0),
                        stop=(c == 7),
                    )
            sf1 = work.tile([H, E], f32)
            nc.vector.tensor_scalar_mul(sf1[:], sf_ps[:], rz[:])
            psC_cm.__exit__(None, None, None)
            sfb = work.tile([H, E], bf16)
            nc.vector.tensor_add(sfb[:], sf1[:], bvb_s[:])

            # sampled[h, d] = sfb[h, h*D + d]: bounce via DRAM with padded rows,
            # then one DRAM->DRAM DMA reads the skewed diagonal view into the
            # contiguous AllGather input (the collective needs a plain AP).
            sf_d = dramp.tile([H, E + D], bf16)
            nc.sync.dma_start(out=sf_d[:, :E], in_=sfb[:])
            sfd_ap = sf_d[:]
            diag_view = bass_mod.AP(
                tensor=sfd_ap.tensor, offset=0, ap=[[E + 2 * D, H], [1, D]]
            )
            s_dram = dramp.tile([1, E], bf16)
            nc.sync.dma_start(
                out=s_dram[:].rearrange("o (h d) -> (o h) d", h=H), in_=diag_view
            )

            # ---- Phase D: AllGather sampled vectors within the TP group ----
            groups = [
                list(range(g * TP_G, (g + 1) * TP_G)) for g in range(NCORES // TP_G)
            ]
            S_all = dramp.tile([TP_G, E], bf16)
            nc.gpsimd.collective_compute(
                "AllGather",
                Alu.bypass,
                replica_groups=groups,
                ins=[s_dram[:].opt()],
                outs=[S_all[:].opt()],
            )

            nc.sync.dma_start(out=S16[0:TP_G, :], in_=S_all[:])
            ST = work.tile([128, 8, H], bf16)
            nc.scalar.dma_start(out=ST[:], in_=S16[:], transpose=True)
            # residual + b2 term only needs S; compute during the MLP matmuls
            sb8 = work.tile([TP_G, E], f32)
            nc.vector.scalar_tensor_tensor(
                sb8[:], S16[0:TP_G, :], 1.0 / TP_G, b28_s[:], Alu.mult, Alu.add
            )

            # ---- Phase E: MLP (tensor-parallel over hidden slice) ----
            psM_cm = tc.tile_pool(name="psM", bufs=1, space="PSUM")
            psM = psM_cm.__enter__()
            h1_ps = psM.tile([TP_G, HID_C], f32, tag="h1")
            for c in range(8):
                for j in range(HID_C // 512):
                    nc.tensor.matmul(
                        h1_ps[:, 512 * j : 512 * (j + 1)],
                        ST[:, c, 0:TP_G],
                        w1_s[:, c, 512 * j : 512 * (j + 1)],
                        start=(c == 0),
                        stop=(c == 7),
                    )
            zb = work.tile([TP_G, HID_C], f32, tag="mza")
            nc.vector.tensor_add(zb[:], h1_ps[:], b1_s[:])
            if GELU_NATIVE:
                nc.scalar.activation(hh16[0:TP_G, :], zb[:], Act.Gelu_apprx_tanh)
            else:
                # gelu_tanh(z) = z * sigmoid(1.5957691...*(z + 0.044715 z^3))
                sq = work.tile([TP_G, HID_C], f32, tag="ga")
                nc.scalar.activation(sq[:], zb[:], Act.Square)
                cb = work.tile([TP_G, HID_C], f32, tag="gb")
                nc.vector.scalar_tensor_tensor(
                    cb[:], sq[:], 0.044715, zb[:], Alu.mult, Alu.mult
                )
                uu = work.tile([TP_G, HID_C], f32, tag="ga")
                nc.vector.tensor_add(uu[:], cb[:], zb[:])
                sg = work.tile([TP_G, HID_C], f32, tag="gb")
                nc.scalar.activation(
                    sg[:], uu[:], Act.Sigmoid, scale=1.5957691216057308
                )
                nc.vector.tensor_mul(hh16[0:TP_G, :], sg[:], zb[:])
            hT = work.tile([128, HID_C // 128, H], bf16)
            nc.scalar.dma_start(out=hT[:], in_=hh16[:], transpose=True)

            p2_ps = psM.tile([TP_G, E], f32, tag="p2")
            for c in range(HID_C // 128):
                for j in range(2):
                    nc.tensor.matmul(
                        p2_ps[:, 512 * j : 512 * (j + 1)],
                        hT[:, c, 0:TP_G],
                        w2_s[:, c, 512 * j : 512 * (j + 1)],
                        start=(c == 0),
                        stop=(c == HID_C // 128 - 1),
                    )
            mlp_s = work.tile([TP_G, E], f32, tag="mza")
            nc.vector.tensor_add(mlp_s[:], p2_ps[:], sb8[:])
            psM_cm.__exit__(None, None, None)
            mlp_d = dramp.tile([TP_G, E], f32)
            nc.sync.dma_start(out=mlp_d[:], in_=mlp_s[:])
            wl_cm.__exit__(None, None, None)

            # ---- Phase F: ReduceScatter -> this core's output row ----
            mlp_row = dramp.tile([1, E], f32)
            nc.gpsimd.collective_compute(
                "ReduceScatter",
                Alu.add,
                replica_groups=groups,
                ins=[mlp_d[:].opt()],
                outs=[mlp_row[:].opt()],
            )
            nc.sync.dma_start(out=out[:, :], in_=mlp_row[:])

    return nc


def get_nc():
    if "nc" not in _CACHE:
        nc = _build()
        nc.finalize()
        _CACHE["nc"] = nc
    return _CACHE["nc"]


def build_in_maps(x, mask, W_kv, b_kv, query, W1, b1, W2, b2):
    """Host-side shard prep. Weight-only algebra + layout transforms."""
    bf = _np_bf16()
    x = np.asarray(x, np.float32)
    mask = np.asarray(mask)
    W_kv = np.asarray(W_kv, np.float32)
    b_kv = np.asarray(b_kv, np.float32)
    query = np.asarray(query, np.float32)
    W1 = np.asarray(W1, np.float32)
    b1 = np.asarray(b1, np.float32)
    W2 = np.asarray(W2, np.float32)
    b2 = np.asarray(b2, np.float32)

    f8 = _np_f8()
    W_k = W_kv[:, :E]
    W_v = W_kv[:, E:]
    # fold the per-head query into the k-projection: [E, H]
    w_att = np.einsum("ehd,hd->eh", W_k.reshape(E, H, D), query).astype(np.float32)
    # [p, c, i, h] with e = c*256 + i*128 + p, scaled x256 for e4m3 range
    wattc = np.ascontiguousarray(
        (w_att * 256.0).reshape(4, 2, 128, H).transpose(2, 0, 1, 3).astype(f8)
    )
    bv_b = np.ascontiguousarray(
        np.broadcast_to(b_kv[None, E:], (H, E)).astype(np.float32)
    )
    Wv_c = np.ascontiguousarray(
        W_v.astype(bf).reshape(8, 128, E).transpose(1, 0, 2)
    )  # [p, c, e]
    b2r = np.ascontiguousarray(
        np.broadcast_to(b2[None, :] / TP_G, (TP_G, E)).astype(np.float32)
    )

    keep = ~mask[:, :, 0]  # [B, N] True = keep
    nmask_ct = (~keep).sum(axis=1).astype(np.float32)  # [B]

    in_maps = []
    for c in range(NCORES):
        r = c % TP_G  # rank within the MLP tensor-parallel group
        hs = slice(HID_C * r, HID_C * (r + 1))
        xm = np.where(keep[c][:, None], x[c], np.float32(0.0))
        xm_bf = xm.astype(bf)
        cpack = np.zeros((H, E + 1 + HID_C + E), dtype=np.float32)
        cpack[:, 0:E] = bv_b
        cpack[:, E] = nmask_ct[c]
        cpack[0:TP_G, E + 1 : E + 1 + HID_C] = b1[hs][None, :]
        cpack[0:TP_G, E + 1 + HID_C :] = b2r
        in_maps.append(
            {
                "xn": np.ascontiguousarray(
                    xm_bf.reshape(NT, 128, E).transpose(1, 0, 2)
                ),  # [p, t, e] with n = t*128 + p
                "xT": np.ascontiguousarray(xm.T.astype(f8)),
                "wattc": wattc,
                "Wv": Wv_c,
                "cpack": cpack,
                "W1c": np.ascontiguousarray(
                    W1[:, hs].astype(bf).reshape(8, 128, HID_C).transpose(1, 0, 2)
                ),
                "W2c": np.ascontiguousarray(
                    W2[hs, :]
                    .astype(bf)
                    .reshape(HID_C // 128, 128, E)
                    .transpose(1, 0, 2)
                ),
            }
        )
    return in_maps


def kernel(**inputs):
    from concourse.bass_utils import run_bass_kernel_spmd

    in_maps = build_in_maps(**inputs)
    nc = get_nc()
    res = run_bass_kernel_spmd(nc, in_maps, list(range(NCORES)), trace=False)
    return np.stack([res.results[c]["out"][0] for c in range(NCORES)]).astype(
        np.float32
    )
